# revision 21
# baseline (speedup 1.0000x reference)
"""MergedEmbeddingBag forward (sum pooling) on 8 Trainium2 NeuronCores.

Strategy (table-parallel, per sharding hint): core t owns table t.

v3 pipeline per core:
  - weights are cast to bf16 on the host and uploaded as [N, 128] bf16, so
    each gathered row is 256 B (half the HBM traffic of fp32) and feeds
    1-pass bf16 matmuls.
  - indices are planned on the host into supergroups of G windows (a window
    = 128 bags); within a supergroup, items are split by table chunk
    (N <= 4 chunks of <= 32767 rows so local indices fit signed int16) and
    gathered with one dma_gather per (supergroup, chunk) - few large SWDGE
    calls (~2.5k indices each, the measured descriptor-throughput sweet
    spot) spread round-robin over all 4 SWDGE queues.  Calls are padded to
    full 128-row slots with row-0 gathers so no SBUF byte a matmul reads is
    ever stale (no startup memset).
  - pooling: for each 128-row slot of gathered data, a one-hot bf16 matrix
    (generated on DVE with a single batched is_equal per call) scatters the
    rows into their bags via a PE matmul accumulated in PSUM.  Slots that
    span window boundaries are matmul'd once per window with masked one-hot
    columns, so there is no per-window padding of the gather stream.
  - scalar (ACT) copies finished PSUM windows to SBUF; sync (HWDGE) stores
    them to HBM.
"""

import sys

sys.path.insert(0, "/opt/trn_rl_repo")

import numpy as np

P = 128  # partitions / bags per window
D = 128  # embedding dim


def _plan3(indices, offsets, n_rows, G=4, max_chunk=32767):
    """Host planning: supergrouped, chunk-split, window-sorted gather order.

    Returns dict with device buffers (idxbuf int16, segbuf bf16 as uint16
    view, iota) and the static schedule consumed by _build_program3.
    """
    import ml_dtypes

    idx64 = np.ascontiguousarray(indices).astype(np.int64)
    off = np.ascontiguousarray(offsets).astype(np.int64)
    t, total = idx64.shape
    b = off.shape[1]
    assert b % P == 0
    n_win = b // P
    assert n_win % G == 0
    n_groups = n_win // G
    n_chunks = -(-n_rows // max_chunk)
    chunk = -(-n_rows // n_chunks)
    assert chunk <= max_chunk

    ends = np.concatenate([off[:, 1:], np.full((t, 1), total, np.int64)], axis=1)
    lens = np.clip(ends - off, 0, None)  # [T, B]

    l_uni = total // b
    uniform = (
        total == b * l_uni
        and (lens == l_uni).all()
        and (off == np.arange(b, dtype=np.int64) * l_uni).all()
    )

    # Per-table flat item lists with window-local bag and window ids, in
    # (window, bag, position) order.
    plans = []
    for i in range(t):
        if uniform:
            vals = idx64[i]  # already (bag, l) order; bags in window order
            bag = (np.arange(total) // l_uni) % P
            win = np.arange(total) // (P * l_uni)
        else:
            ls = lens[i]
            bag_of_item = np.repeat(np.arange(b), ls)
            pos = np.concatenate(
                [np.arange(off[i, j], off[i, j] + ls[j]) for j in range(b)]
            ) if ls.sum() else np.zeros(0, np.int64)
            vals = idx64[i, pos]
            bag = bag_of_item % P
            win = bag_of_item // P
        plans.append((vals, bag, win))

    # Shared static schedule across tables: per (group, chunk) call sizes
    # must be identical for the single SPMD program, so take the max and
    # pad with -1 (skipped).  mm schedule must also be shared: a matmul
    # exists for (call, slot, window) if ANY table has items there; its
    # seg column is per-table data.
    calls = []  # static: per call dict
    idx_cols = []  # per call: [T, p16] int16 (pad -1)
    seg_cols = []  # per call: [T, n_mm, 128] float (pad -1)
    for g in range(n_groups):
        w0, w1 = g * G, (g + 1) * G
        for c in range(n_chunks):
            per_t = []
            for i in range(t):
                vals, bag, win = plans[i]
                m = (win >= w0) & (win < w1) & (vals >= c * chunk) & (
                    vals < (c + 1) * chunk
                )
                per_t.append(
                    (
                        (vals[m] - c * chunk).astype(np.int16),
                        bag[m].astype(np.int16),
                        (win[m] - w0).astype(np.int16),
                    )
                )
            nmax = max(len(v) for v, _, _ in per_t)
            nmax = max(nmax, 1)
            # pad to FULL slots (pads rewritten to row 0 later): every gbuf
            # byte that a matmul can read gets written by its own call, so no
            # stale-NaN risk and no startup memset is needed.
            n_slots = -(-nmax // P)
            p16 = n_slots * P
            iv = np.full((t, p16), -1, np.int16)
            bv = np.full((t, n_slots * P), -1, np.int16)
            wv = np.full((t, n_slots * P), -2, np.int16)
            nvalid = np.zeros(t, np.int64)
            for i in range(t):
                v, bg, wn = per_t[i]
                k = len(v)
                iv[i, :k] = v
                bv[i, :k] = bg
                wv[i, :k] = wn
                nvalid[i] = k
            # mm list: per slot, windows present in ANY table (ascending)
            mms = []
            segs = []
            for s in range(n_slots):
                sl = slice(s * P, (s + 1) * P)
                wins_here = np.unique(wv[:, sl])
                wins_here = wins_here[wins_here >= 0]
                for wloc in wins_here:
                    seg = np.where(wv[:, sl] == wloc, bv[:, sl], -1).astype(
                        np.float32
                    )  # [T, 128]
                    mms.append((s, int(wloc)))
                    segs.append(seg)
            calls.append(
                dict(
                    group=g,
                    chunk=c,
                    p16=p16,
                    n_slots=n_slots,
                    nvalid=nvalid,
                    mms=mms,
                )
            )
            idx_cols.append(iv)
            seg_cols.append(
                np.stack(segs, axis=1) if segs else np.zeros((t, 0, P), np.float32)
            )

    # start/stop flags per matmul: per group, first/last mm of each window.
    n_per_group = len(calls) // n_groups
    for g in range(n_groups):
        order = []  # (call_idx, mm_idx, wloc)
        for cc in range(g * n_per_group, (g + 1) * n_per_group):
            for mi, (s, wloc) in enumerate(calls[cc]["mms"]):
                order.append((cc, mi, wloc))
        firsts, lasts = {}, {}
        for k, (cc, mi, wloc) in enumerate(order):
            if wloc not in firsts:
                firsts[wloc] = (cc, mi)
            lasts[wloc] = (cc, mi)
        for cc in range(g * n_per_group, (g + 1) * n_per_group):
            flags = []
            for mi, (s, wloc) in enumerate(calls[cc]["mms"]):
                flags.append(
                    (
                        firsts[wloc] == (cc, mi),
                        lasts[wloc] == (cc, mi),
                    )
                )
            calls[cc]["flags"] = flags
        # every window in the group must have at least one mm (else its
        # psum region is never written); guaranteed here because every
        # window has >= 1 item in >= 1 chunk.  Guard anyway:
        assert len(firsts) == G or b == 0, (g, sorted(firsts))

    # Device buffers.
    # idxbuf: concat per-call [p16] wrapped to [16, p16/16], tiled to 128.
    iparts = []
    for iv in idx_cols:
        p16 = iv.shape[1]
        iparts.append(iv.reshape(t, p16 // 16, 16).transpose(0, 2, 1))
    idxbuf16 = np.concatenate(iparts, axis=2)  # [T, 16, IC]
    idxbuf = np.ascontiguousarray(np.tile(idxbuf16, (1, 8, 1)))  # [T, 128, IC]

    # segbuf: [T, 128, M_total] bf16 (partition p = slot row)
    sparts = [sc.transpose(0, 2, 1) for sc in seg_cols]  # [T, 128, n_mm]
    segbuf = np.concatenate(sparts, axis=2).astype(ml_dtypes.bfloat16)

    iota = np.tile(
        np.arange(P, dtype=np.float32)[None, :], (P, 1)
    ).astype(ml_dtypes.bfloat16)

    return dict(
        calls=calls,
        idxbuf=idxbuf,
        segbuf=np.ascontiguousarray(segbuf),
        iota=iota,
        chunk=chunk,
        n_chunks=n_chunks,
        G=G,
        n_groups=n_groups,
        n_win=n_win,
    )


def _build_program3(n_rows, plan, nbuf=2, ohb=10, o_bufs=4, scratch=16384):
    """Raw-Bass SPMD program for the v3 pipeline."""
    import contextlib

    import concourse.bass as bass
    import concourse.mybir as mybir
    from concourse import library_config

    calls = plan["calls"]
    chunk = plan["chunk"]
    G = plan["G"]
    n_groups = plan["n_groups"]
    n_win = plan["n_win"]
    n_calls = len(calls)
    n_per_group = n_calls // n_groups
    ic = plan["idxbuf"].shape[2]
    sc = max(1, plan["segbuf"].shape[2])

    # static derived counts
    call_mm = [len(c["mms"]) for c in calls]
    mm_cum = np.cumsum([0] + call_mm)  # mm count before call j
    group_slots = [
        sum(calls[cc]["n_slots"] for cc in range(g * n_per_group, (g + 1) * n_per_group))
        for g in range(n_groups)
    ]
    region_slots = max(group_slots)
    call_mm_max = max(call_mm)
    # group gbuf slot offsets per call
    call_goff = []
    for g in range(n_groups):
        o = 0
        for cc in range(g * n_per_group, (g + 1) * n_per_group):
            call_goff.append(o)
            o += calls[cc]["n_slots"]
    q_of_call = [j % 4 for j in range(n_calls)]
    # mm index of each window's last matmul (global, in issue order)
    last_mm_of_win = {}
    k = 0
    for j, c in enumerate(calls):
        g = c["group"]
        for (s, wloc) in c["mms"]:
            last_mm_of_win[g * G + wloc] = k
            k += 1

    bf16 = mybir.dt.bfloat16
    f32 = mybir.dt.float32

    nc = bass.Bass(num_swdge_queues=4, dynamic_dma_scratch_size=scratch)
    wz = nc.declare_dram_parameter("wz", [n_rows, D], bf16, isOutput=False)
    idx = nc.declare_dram_parameter("idx", [P, ic], mybir.dt.int16, isOutput=False)
    seg = nc.declare_dram_parameter("seg", [P, sc], bf16, isOutput=False)
    iota = nc.declare_dram_parameter("iota", [P, P], bf16, isOutput=False)
    out = nc.declare_dram_parameter("out", [n_win * P, D], f32, isOutput=True)

    with contextlib.ExitStack() as ctx:
        idx_sb = ctx.enter_context(nc.sbuf_tensor([P, ic], mybir.dt.int16))
        seg_sb = ctx.enter_context(nc.sbuf_tensor([P, sc], bf16))
        iota_sb = ctx.enter_context(nc.sbuf_tensor([P, P], bf16))
        gbuf = ctx.enter_context(
            nc.sbuf_tensor([P, nbuf * region_slots * D], bf16)
        )
        ohbuf = ctx.enter_context(
            nc.sbuf_tensor([P, ohb * call_mm_max * P], bf16)
        )
        obuf = ctx.enter_context(nc.sbuf_tensor([P, o_bufs * D], f32))
        # One PSUM BANK per in-flight window: a matmul's start=True resets
        # the whole bank, so windows must not share banks while accumulating.
        assert G <= 4
        psums = [
            ctx.enter_context(nc.psum_tensor(f"ps{i}", [P, P], f32))
            for i in range(2 * G)
        ]
        in_sem = ctx.enter_context(nc.semaphore("in_sem"))
        isem = ctx.enter_context(nc.semaphore("isem"))
        # One gather-completion sem per (region, call-in-group) so at most
        # ONE DMA is ever in flight per sem (per-engine increments from two
        # concurrent DMAs interleave, so cumulative ge-16k waits on a shared
        # sem are racy).  Region gating (mmsem) bounds in-flight per sem to 1.
        n_gsem = nbuf * n_per_group
        gsems = [ctx.enter_context(nc.semaphore(f"gsem{i}")) for i in range(n_gsem)]
        ohsem = ctx.enter_context(nc.semaphore("ohsem"))
        mmsem = ctx.enter_context(nc.semaphore("mmsem"))
        csem = ctx.enter_context(nc.semaphore("csem"))
        ssems = [ctx.enter_context(nc.semaphore(f"ssem{i}")) for i in range(o_bufs)]
        block = ctx.enter_context(nc.Block())

        def gslot(j, s):
            """SBUF tile [P, D] of slot s of call j."""
            g = calls[j]["group"]
            base = (g % nbuf) * region_slots * D + (call_goff[j] + s) * D
            return gbuf[:, base : base + D]

        def gdest(j):
            g = calls[j]["group"]
            base = (g % nbuf) * region_slots * D + call_goff[j] * D
            n_slots = calls[j]["n_slots"]
            return gbuf[:, base : base + n_slots * D].rearrange(
                "p (s e) -> p s e", e=D
            )

        def ohcol(m_global, j):
            r = j % ohb
            off = (m_global - mm_cum[j]) * P
            return ohbuf[:, r * call_mm_max * P + off : r * call_mm_max * P + off + P]

        # idx columns needed by the first nbuf groups' calls (prefix load
        # lets gathers start before the full idx buffer has landed)
        icol_pref = sum(calls[j]["p16"] // 16 for j in range(min(nbuf * n_per_group, n_calls)))

        @block.sync
        def _(sync):
            sync.dma_start(idx_sb[:, :icol_pref], idx[:, :icol_pref]).then_inc(isem, 16)
            if icol_pref < ic:
                sync.dma_start(idx_sb[:, icol_pref:], idx[:, icol_pref:]).then_inc(isem, 16)
            sync.dma_start(seg_sb[:], seg[:]).then_inc(in_sem, 16)
            sync.dma_start(iota_sb[:], iota[:]).then_inc(in_sem, 16)
            for w in range(n_win):
                sync.wait_ge(csem, w + 1)
                sync.dma_start(
                    out[w * P : (w + 1) * P, :],
                    obuf[:, (w % o_bufs) * D : (w % o_bufs + 1) * D],
                ).then_inc(ssems[w % o_bufs], 16)
            for lane in range(o_bufs):
                n_l = len(range(lane, n_win, o_bufs))
                if n_l:
                    sync.wait_ge(ssems[lane], 16 * n_l)

        @block.gpsimd
        def _(g):
            g.load_library(library_config.mlp)
            g.wait_ge(isem, 16)
            full_idx_waited = icol_pref >= ic
            reg_ctx = g.register("ni_reg")
            ni = reg_ctx.__enter__()
            icol = 0
            for j, c in enumerate(calls):
                grp = c["group"]
                if not full_idx_waited and j >= nbuf * n_per_group:
                    g.wait_ge(isem, 32)
                    full_idx_waited = True
                if j % n_per_group == 0 and grp >= nbuf:
                    g.wait_ge(mmsem, int(mm_cum[(grp - nbuf + 1) * n_per_group]))
                # nvalid differs per table but the SPMD program is shared,
                # so pads point at row 0 (valid) and every core gathers p16.
                g.reg_mov(ni, c["p16"])
                g.dma_gather(
                    out_ap=gdest(j),
                    in_ap=wz[c["chunk"] * chunk : min((c["chunk"] + 1) * chunk, n_rows), :],
                    idxs_ap=idx_sb[:, icol : icol + c["p16"] // 16],
                    num_idxs=c["p16"],
                    num_idxs_reg=ni,
                    elem_size=D,
                    single_packet=False,
                    queue_num=q_of_call[j],
                ).then_inc(gsems[j % n_gsem], 16)
                icol += c["p16"] // 16

        @block.vector
        def _(v):
            v.wait_ge(in_sem, 32)
            for j, c in enumerate(calls):
                n_mm = call_mm[j]
                if n_mm == 0:
                    continue
                if j >= ohb:
                    v.wait_ge(mmsem, int(mm_cum[j - ohb + 1]))
                r = j % ohb
                o = ohbuf[
                    :, r * call_mm_max * P : r * call_mm_max * P + n_mm * P
                ].rearrange("p (m e) -> p m e", e=P)
                s_in = (
                    seg_sb[:, mm_cum[j] : mm_cum[j + 1]]
                    .rearrange("p (m o) -> p m o", o=1)
                    .broadcast_to([P, n_mm, P])
                )
                i_in = (
                    iota_sb[:]
                    .rearrange("p (o e) -> p o e", o=1)
                    .broadcast_to([P, n_mm, P])
                )
                v.tensor_tensor(
                    out=o, in0=s_in, in1=i_in, op=mybir.AluOpType.is_equal
                ).then_inc(ohsem, 1)

        @block.tensor
        def _(pe):
            m_global = 0
            for j, c in enumerate(calls):
                grp = c["group"]
                if j % n_per_group == 0 and grp >= 2:
                    # psum region (grp % 2) free when group grp-2 fully copied
                    pe.wait_ge(csem, (grp - 1) * G)
                pe.wait_ge(gsems[j % n_gsem], 16 * (j // n_gsem + 1))
                if call_mm[j]:
                    pe.wait_ge(ohsem, sum(1 for jj in range(j + 1) if call_mm[jj]))
                for mi, (s, wloc) in enumerate(c["mms"]):
                    st, sp = c["flags"][mi]
                    pe.matmul(
                        psums[(grp % 2) * G + wloc][:],
                        lhsT=ohcol(m_global, j),
                        rhs=gslot(j, s),
                        start=st,
                        stop=sp,
                        skip_group_check=True,
                    ).then_inc(mmsem, 1)
                    m_global += 1

        @block.scalar
        def _(a):
            for w in range(n_win):
                a.wait_ge(mmsem, int(last_mm_of_win[w]) + 1)
                if w >= o_bufs:
                    wp = w - o_bufs
                    a.wait_ge(ssems[wp % o_bufs], 16 * (wp // o_bufs + 1))
                grp = w // G
                wloc = w % G
                a.copy(
                    obuf[:, (w % o_bufs) * D : (w % o_bufs + 1) * D],
                    psums[(grp % 2) * G + wloc][:],
                ).then_inc(csem, 1)

    return nc


def _run(weights, indices, offsets, trace=False, G=4, scratch=16384):
    import ml_dtypes
    from concourse import mybir
    from concourse.bass_utils import run_bass_kernel_spmd

    weights = np.asarray(weights)
    t, n, d = weights.shape
    assert d == D

    b = np.asarray(offsets).shape[1]
    n_win = b // P
    while G > 1 and n_win % G:
        G -= 1
    plan = _plan3(indices, offsets, n, G=G)

    # pads must gather a real row (see note in gpsimd block): rewrite -1
    # pads in idxbuf to 0.
    idxbuf = plan["idxbuf"].copy()
    idxbuf[idxbuf < 0] = 0

    wz16 = weights.astype(ml_dtypes.bfloat16)

    nc = _build_program3(n, plan, scratch=scratch)
    mybir.codegen_inst_isa_subclasses(nc)
    in_maps = [
        {
            "wz": np.ascontiguousarray(wz16[i]),
            "idx": np.ascontiguousarray(idxbuf[i]),
            "seg": np.ascontiguousarray(plan["segbuf"][i]),
            "iota": plan["iota"],
        }
        for i in range(t)
    ]
    res = run_bass_kernel_spmd(nc, in_maps, list(range(t)), trace=trace)
    out = np.stack([res.results[i]["out"] for i in range(t)], axis=0)
    return out, res


def kernel(weights, indices, offsets):
    out, _ = _run(weights, indices, offsets)
    return out
